# revision 9
# baseline (speedup 1.0000x reference)
"""Trainium2 Bass kernel for nn_MemoryCell (scatter_memory).

Full-input contract: kernel(**inputs) takes the complete (unsharded) numpy
inputs and returns the full [NB*B, H] output.

Math (B == H == 1024, NB == 5, T == 128):
    enc  = features[:, 0, :]                         # [B, H] - only slice used
    h    = states.reshape(NB, H)
    gate = sigmoid(enc @ (h + keys).T)               # [B, NB]
    pre  = (h @ Uw.T + keys @ Vw.T)[:, None, :] + (enc @ Ww.T)[None, :, :]
    cand = where(pre >= 0, pre, prelu_a * pre)
    new[i, b, j] = h[i, j] + gate[j, i] * cand[i, b, j]   # B==H broadcast quirk
    out  = sign(new) with exact zeros -> +1, reshaped [NB*B, H]

Sharding: split the feature/column axis j (H=1024) into 8 shards of 128
(one per core).  Each core computes ew = enc @ Ww[js].T for all b plus its
own gate[js]/huv[js] slices, so nothing needs a collective.

Precision: enc ships HI-ONLY fp16 (2 MB/core) and the big ew matmul is a
SINGLE fp16 pass (e_hi @ W_hi) - the sign-output tolerance (rel < 2e-2 ~=
524 sign flips) makes the ~6e-4 product error harmless (~140 flips
against an fp64 oracle).  Only the gate logits z keep extra passes
(e_hi @ hk_lo and e_lo[js] @ hk_hi) since sigmoid sensitivity near z~0
is the dominant flip source.

The enc columns are rolled per-core so each core's own 128 j-columns sit
first: the z-series moving operand is then a plain [0:128] slice of the
main enc stream (no separate tensor, same program on every core), and the
host un-rolls the output columns.

Tail: sign(gate*ew + gate*huv + h) == (ew >= c) with
c = -(gate*huv + h) / max(gate, 1e-30), a per-partition scalar -> ONE
compare op per (i, quarter): DVE takes i=0,1,2 as is_ge -> int8 {1,0},
ScalarE takes i=3,4 as ACT Sign (scale=gate, bias=gate*huv+h) -> int8
{-1,0,1}.  gate==0 underflow reproduces sign(h) exactly through the
clamp.  The host re-expands int8 to +-1.0 fp32.
"""

import os
import numpy as np

H = 1024
NB = 5
B = 1024
NCORES = 8
JS = H // NCORES          # 128 columns per core
KC = H // 128             # 8 contraction chunks
NQ = 4                    # b processed in quarters (PSUM + tail pipelining)
QB = B // NQ
SWH = 69                  # S_hi width: hk@0, h@32, keys@64
SWL = 5                   # S_lo width: hk_lo@0
N_WARM = 4                # PE clock-ramp transposes before real work

_NC_CACHE = {}


def _build_nc(general_prelu: bool):
    from concourse import bacc, mybir
    import concourse.tile as tile

    f32 = mybir.dt.float32
    f16 = mybir.dt.float16
    i8 = mybir.dt.int8
    AF = mybir.ActivationFunctionType
    ALU = mybir.AluOpType

    nc = bacc.Bacc("TRN2", debug=False, num_devices=NCORES)

    # enc.T hi fp16, b-columns rolled so this core's js sit at 0:128.
    eha = nc.dram_tensor("eha", [128, KC, B // 2], f16,
                         kind="ExternalInput").ap()
    ehc = nc.dram_tensor("ehc", [128, KC, B // 2], f16,
                         kind="ExternalInput").ap()
    elj = nc.dram_tensor("elj", [128, KC, JS], f16, kind="ExternalInput").ap()
    ut = nc.dram_tensor("ut", [128, KC, JS], f16, kind="ExternalInput").ap()
    vt = nc.dram_tensor("vt", [128, KC, JS], f16, kind="ExternalInput").ap()
    wt = nc.dram_tensor("wt", [128, KC, JS], f16, kind="ExternalInput").ap()
    ssh = nc.dram_tensor("ssh", [128, KC, SWH], f16, kind="ExternalInput").ap()
    ssl = nc.dram_tensor("ssl", [128, KC, SWL], f16, kind="ExternalInput").ap()
    # hsm cols: 0:5 h[:,js].T, 5:6 prelu_a[js] (only when general_prelu)
    hs_f = NB + (1 if general_prelu else 0)
    hsm = nc.dram_tensor("hsm", [128, hs_f], f32, kind="ExternalInput").ap()
    idn = nc.dram_tensor("idn", [128, 128], f32, kind="ExternalInput").ap()
    out = nc.dram_tensor("out", [128, NB, B], i8, kind="ExternalOutput").ap()

    with tile.TileContext(nc) as tc:
        with (
            tc.tile_pool(name="res", bufs=1) as res,
            tc.tile_pool(name="pew", bufs=2, space="PSUM") as pew,
            tc.tile_pool(name="psmall", bufs=1, space="PSUM") as psmall,
        ):
            # ---- input DMAs: 3 issue queues (sync / scalar / gpsimd) ----
            eh_t = [res.tile([128, KC, B // 2], f16, name=f"eh_t{hf}",
                             tag=f"eh{hf}") for hf in range(2)]
            nc.sync.dma_start(eh_t[0], eha)
            nc.sync.dma_start(eh_t[1], ehc)

            idn_t = res.tile([128, 128], f32, name="idn_t")
            nc.scalar.dma_start(idn_t, idn)
            ssh_t = res.tile([128, KC, SWH], f16, name="ssh_t")
            nc.scalar.dma_start(ssh_t, ssh)
            ssl_t = res.tile([128, KC, SWL], f16, name="ssl_t")
            nc.scalar.dma_start(ssl_t, ssl)
            wt_t = res.tile([128, KC, JS], f16, name="wt_t")
            nc.scalar.dma_start(wt_t, wt)
            hsm_t = res.tile([128, hs_f], f32, name="hsm_t")
            nc.scalar.dma_start(hsm_t, hsm)

            elj_t = res.tile([128, KC, JS], f16, name="elj_t")
            nc.gpsimd.dma_start(elj_t, elj)
            ut_t = res.tile([128, KC, JS], f16, name="ut_t")
            nc.gpsimd.dma_start(ut_t, ut)
            vt_t = res.tile([128, KC, JS], f16, name="vt_t")
            nc.gpsimd.dma_start(vt_t, vt)

            # ---- PE ramp warm-up on the identity ----
            psum_warm = psmall.tile([128, 128], f32, name="psum_warm")
            for _ in range(N_WARM):
                nc.tensor.transpose(psum_warm, idn_t, idn_t)

            # ---- small series: z / hu / kv in one [SWH, 384] PSUM tile ----
            # col 0:128 = z terms vs e[js], 128:256 = hu vs Uw[js],
            # 256:384 = kv vs Vw[js]; garbage off-blocks ignored.
            psum_gv = psmall.tile([SWH, 3 * JS], f32, name="psum_gv")
            zb = psum_gv[:, 0:JS]
            hb = psum_gv[:, JS:2 * JS]
            kb = psum_gv[:, 2 * JS:3 * JS]
            ehj = eh_t[0][:, :, 0:JS]
            zb5 = psum_gv[0:SWL, 0:JS]
            # NOTE: start=True clears the WHOLE psum tile (not just the
            # written region) - exactly one start (first MM) and one stop
            # (last MM) for the entire interleaved series.
            for k in range(KC):
                s_hi = ssh_t[:, k, :]
                first, last = (k == 0), (k == KC - 1)
                if not first:
                    # S_lo leads; 2 LDWEIGHTS per chunk
                    nc.tensor.matmul(zb5, lhsT=ssl_t[:, k, :],
                                     rhs=ehj[:, k, :], start=False, stop=False)
                nc.tensor.matmul(zb, lhsT=s_hi, rhs=ehj[:, k, :],
                                 start=first, stop=False)
                nc.tensor.matmul(hb, lhsT=s_hi, rhs=ut_t[:, k, :],
                                 start=False, stop=False)
                nc.tensor.matmul(kb, lhsT=s_hi, rhs=vt_t[:, k, :],
                                 start=False, stop=False)
                nc.tensor.matmul(zb, lhsT=s_hi, rhs=elj_t[:, k, :],
                                 start=False, stop=last)
                if first:
                    nc.tensor.matmul(zb5, lhsT=ssl_t[:, k, :],
                                     rhs=ehj[:, k, :], start=False, stop=False)

            # pack z/hu/kv rows into one tile, transpose to j-on-partitions
            gh_sb = res.tile([128, 128], f32, name="gh_sb")
            nc.gpsimd.memset(gh_sb, 0.0)
            nc.vector.tensor_copy(out=gh_sb[0:NB, :], in_=psum_gv[0:NB, 0:JS])
            nc.vector.tensor_copy(out=gh_sb[32:32 + NB, :],
                                  in_=psum_gv[32:32 + NB, JS:2 * JS])
            nc.vector.tensor_copy(out=gh_sb[64:64 + NB, :],
                                  in_=psum_gv[64:64 + NB, 2 * JS:3 * JS])

            # ---- ew = e_hi @ Ww[js]_hi.T, one fp16 pass, b in quarters ----
            o_all = res.tile([128, NB, B], i8, name="o_all")
            vecs = gate = bias3 = c_sb = huv = None
            for q in range(NQ):
                pew_t = pew.tile([128, QB], f32, name="pew_t", tag="ew")
                src = eh_t[q // 2]
                lo = (q % 2) * QB
                for k in range(KC):
                    nc.tensor.matmul(pew_t, lhsT=wt_t[:, k, :],
                                     rhs=src[:, k, lo:lo + QB],
                                     start=(k == 0), stop=(k == KC - 1))
                if q == 0:
                    # transpose gh between ew quarters; the tiny DVE chain
                    # builds gate / bias3 / c while quarter 1 streams
                    psum_gh = psmall.tile([128, 128], f32, name="psum_gh")
                    nc.tensor.transpose(psum_gh, gh_sb, idn_t)
                    # vecs cols: 0:5 gate, 5:10 huv, 10:15 bias3, 15:20 gcl,
                    # 20:25 rec, 25:30 c
                    vecs = res.tile([128, 30], f32, name="vecs")
                    gate = vecs[:, 0:NB]
                    huv = vecs[:, 5:5 + NB]
                    bias3 = vecs[:, 10:10 + NB]
                    gcl = vecs[:, 15:15 + NB]
                    rec = vecs[:, 20:20 + NB]
                    c_sb = vecs[:, 25:25 + NB]
                    nc.scalar.activation(gate, psum_gh[:, 0:NB], AF.Sigmoid)
                    nc.vector.tensor_copy(out=huv, in_=psum_gh[:, 32:32 + NB])
                    nc.vector.tensor_tensor(huv, huv,
                                            psum_gh[:, 64:64 + NB], ALU.add)
                    nc.vector.tensor_tensor(bias3, gate, huv, ALU.mult)
                    nc.vector.tensor_tensor(bias3, bias3, hsm_t[:, 0:NB],
                                            ALU.add)
                    nc.vector.tensor_scalar_max(gcl, gate, 1e-30)
                    nc.vector.reciprocal(rec, gcl)
                    nc.vector.scalar_tensor_tensor(
                        c_sb, in0=bias3, scalar=-1.0, in1=rec,
                        op0=ALU.mult, op1=ALU.mult)
                for i in range(NB):
                    dst = o_all[:, i, q * QB:(q + 1) * QB]
                    if general_prelu:
                        # generic PReLU path (prelu_a != 1): rebuild cand
                        a_col = hsm_t[:, NB:NB + 1]
                        pre = res.tile([128, QB], f32, name="pre", tag="pre",
                                       bufs=2)
                        nc.vector.tensor_scalar_add(pre, pew_t, huv[:, i:i + 1])
                        mx = res.tile([128, QB], f32, name="mx", tag="mx",
                                      bufs=2)
                        nc.vector.tensor_scalar_max(mx, pre, 0.0)
                        mn = res.tile([128, QB], f32, name="mn", tag="mn",
                                      bufs=2)
                        nc.vector.tensor_scalar_min(mn, pre, 0.0)
                        cand = res.tile([128, QB], f32, name="cand", tag="cand",
                                        bufs=2)
                        nc.vector.scalar_tensor_tensor(
                            cand, in0=mn, scalar=a_col, in1=mx,
                            op0=ALU.mult, op1=ALU.add)
                        nc.scalar.activation(
                            dst, cand, AF.Sign, bias=hsm_t[:, i:i + 1],
                            scale=gate[:, i:i + 1])
                    elif i >= 3:
                        nc.scalar.activation(
                            dst, pew_t, AF.Sign, bias=bias3[:, i:i + 1],
                            scale=gate[:, i:i + 1])
                    else:
                        nc.vector.tensor_scalar(
                            dst, pew_t, c_sb[:, i:i + 1], None, ALU.is_ge)
                if q == 1:
                    nc.sync.dma_start(out[:, :, 0:B // 2],
                                      o_all[:, :, 0:B // 2])
                elif q == NQ - 1:
                    nc.sync.dma_start(out[:, :, B // 2:B],
                                      o_all[:, :, B // 2:B])

    nc.compile()
    return nc


def _get_nc(general_prelu: bool):
    nc = _NC_CACHE.get(general_prelu)
    if nc is None:
        nc = _build_nc(general_prelu)
        _NC_CACHE[general_prelu] = nc
    return nc


def _c32(a):
    return np.ascontiguousarray(a, dtype=np.float32)


def _packT(mat_t):
    # [H, F] (contraction-major rows) -> [128, KC, F]
    F = mat_t.shape[1]
    return np.ascontiguousarray(
        mat_t.reshape(KC, 128, F).transpose(1, 0, 2))


def _split16(a):
    hi = a.astype(np.float16)
    lo = (a - hi.astype(np.float32)).astype(np.float16)
    return hi, lo


def kernel(features, states, Uw, Vw, Ww, keys, prelu_a):
    from concourse import bass_utils

    features = np.asarray(features)
    states = np.asarray(states, dtype=np.float32)
    Uw = np.asarray(Uw, dtype=np.float32)
    Vw = np.asarray(Vw, dtype=np.float32)
    Ww = np.asarray(Ww, dtype=np.float32)
    keys = np.asarray(keys, dtype=np.float32)
    prelu_a = np.asarray(prelu_a, dtype=np.float32)

    enc = np.ascontiguousarray(features[:, 0, :], dtype=np.float32)  # [B, H]
    h = states.reshape(NB, H)
    hk = h + keys

    general_prelu = not np.all(prelu_a == 1.0)
    nc = _get_nc(general_prelu)

    e_hi, e_lo = _split16(enc)
    ehT = _packT(np.ascontiguousarray(e_hi.T))       # [128, KC, B] f16
    elT = _packT(np.ascontiguousarray(e_lo.T))

    hk_hi, hk_lo = _split16(hk)
    h_hi, _ = _split16(h)
    k_hi, _ = _split16(keys)
    sshA = np.zeros((128, KC, SWH), dtype=np.float16)
    sshA[:, :, 0:NB] = _packT(hk_hi.T)
    sshA[:, :, 32:32 + NB] = _packT(h_hi.T)
    sshA[:, :, 64:64 + NB] = _packT(k_hi.T)
    sslA = np.ascontiguousarray(_packT(hk_lo.T))

    idnA = np.eye(128, dtype=np.float32)

    in_maps = []
    for c in range(NCORES):
        js = slice(c * JS, (c + 1) * JS)
        ehR = np.roll(ehT, -c * JS, axis=2)          # own js columns first
        hs_parts = [_c32(h[:, js].T)]
        if general_prelu:
            hs_parts.append(_c32(prelu_a[js].reshape(JS, 1)))
        in_maps.append({
            "eha": np.ascontiguousarray(ehR[:, :, 0:B // 2]),
            "ehc": np.ascontiguousarray(ehR[:, :, B // 2:]),
            "elj": np.ascontiguousarray(elT[:, :, js]),
            "ut": _packT(Uw[js].T.astype(np.float16)),
            "vt": _packT(Vw[js].T.astype(np.float16)),
            "wt": _packT(Ww[js].T.astype(np.float16)),
            "ssh": sshA,
            "ssl": sslA,
            "hsm": np.ascontiguousarray(np.concatenate(hs_parts, axis=1)),
            "idn": idnA,
        })

    trace = bool(int(os.environ.get("KERNEL_TRACE", "0")))
    res = bass_utils.run_bass_kernel_spmd(
        nc, in_maps, core_ids=list(range(NCORES)), trace=trace)
    kernel.last_result = res

    one = np.float32(1.0)
    neg = np.float32(-1.0)
    full = np.empty((NB, B, H), dtype=np.float32)
    view = full.reshape(NB, B, NCORES, JS)
    for c in range(NCORES):
        oc = res.results[c]["out"]                   # [128, NB, B-rolled]
        oc = np.roll(oc, c * JS, axis=2)             # un-roll b columns
        oc = oc.transpose(1, 2, 0)                   # [NB, B, 128]
        # i 0-2: DVE is_ge {1,0}: >0 -> +1
        # i 3-4 (and all i under general_prelu): ACT Sign {-1,0,1}: >=0 -> +1
        if general_prelu:
            view[:, :, c, :] = np.where(oc >= 0, one, neg)
        else:
            view[:, :, c, :][0:3] = np.where(oc[0:3] > 0, one, neg)
            view[:, :, c, :][3:NB] = np.where(oc[3:NB] >= 0, one, neg)
    return full.reshape(NB * B, H)
